# revision 9
# baseline (speedup 1.0000x reference)
"""DGD memory-update kernel for Trainium2 (8 NeuronCores, SPMD).

Computes, per (b, h) pair:
    pred = M @ k                    # [D,1] matvec
    err  = pred - v
    out  = alpha * M + eta * err @ k^T

Sharding: batch dim B=16 split across 8 cores (2 b x 16 h = 32 pairs/core).
Zero inter-core communication.

Per-pair dataflow on one core (D=512 as 4 row-chunks on 128 partitions,
partition-major layout: SBUF tile[p, c*512+j] = M[4*p+c, j] so each
partition's DMA run is 8KB contiguous):
  - k broadcast across partitions: one strided DMA plants k at partitions
    {0,32,64,96}, one DVE stream_shuffle(mask=[0]*32) replicates lane 0 of
    each 32-group to the whole group.  (Custom gpsimd/PE broadcasts are
    broken on this stack; this path is exact and verified.)
  - DVE tensor_tensor mult (one batched [128,4,512] op, k broadcast along
    the chunk axis via a stride-0 AP) -> tmp.
  - ACT activation(Copy, accum_out) row-reduces tmp per chunk -> pred
    (free-axis accumulate on the scalar engine).
  - s_err = eta*(pred - v) on [128,4] (tiny DVE ops).
  - ACT pre-scales Ms = alpha*M (per-partition scale AP, one batched op).
  - DVE tensor_scalar builds outer[:,c,:] = k_bc * s_err[:,c].
  - GPSIMD tensor_tensor add (one batched op): out = Ms + outer.
HBM traffic is the 2MB/pair floor (M in + out), so the kernel is DMA-bound
(~6.1us/pair at ~350GB/s/core aggregate DMA).

Engine budget per pair (est): DMA 6.1us | ACT 4.9us | DVE 4.5us |
GPSIMD 4.3us | SP-seq 2.3us.
"""

import numpy as np

B, H, D = 16, 16, 512
N_CORES = 8
B_PER_CORE = B // N_CORES            # 2
PAIRS_PER_CORE = B_PER_CORE * H      # 32
P = 128                              # SBUF partitions
C = D // P                           # 4 row-chunks per pair

_CACHE = {}


def _legalize_sync_waits(nc, mybir, max_waits=1):
    # The walrus build in this container rejects instructions carrying more
    # than one sync wait; hoist excess waits onto preceding same-engine NOPs.
    for f in nc.m.functions:
        for bb in f.blocks:
            out = []
            for inst in bb.instructions:
                si = inst.sync_info
                if si is not None and si.on_wait and len(si.on_wait) > max_waits:
                    waits = list(si.on_wait)
                    excess, keep = waits[:-max_waits], waits[-max_waits:]
                    for i in range(0, len(excess), max_waits):
                        nop = mybir.InstNoOp(
                            name=nc.get_next_instruction_name(),
                            engine=inst.engine,
                            ins=[],
                            outs=[],
                            bass_nofuse=True,
                            sync_info=mybir.SyncInfo(
                                on_wait=excess[i : i + max_waits], on_update=[]
                            ),
                        )
                        nc.register_instruction(nop)
                        out.append(nop)
                    si.on_wait = keep
                out.append(inst)
            bb.instructions[:] = out


def _build_program():
    import concourse.bass as bass
    import concourse.tile as tile
    from concourse import mybir

    f32 = mybir.dt.float32
    mult = mybir.AluOpType.mult
    add = mybir.AluOpType.add
    subtract = mybir.AluOpType.subtract
    Copy = mybir.ActivationFunctionType.Copy
    BCAST32 = [0] * 32  # stream_shuffle mask: every lane reads lane 0 of its 32-group

    nc = bass.Bass()
    mem_ext = nc.dram_tensor("memory", [B_PER_CORE, H, D, D], f32, kind="ExternalInput")
    k_ext = nc.dram_tensor("k", [B_PER_CORE, H, D, 1], f32, kind="ExternalInput")
    v_ext = nc.dram_tensor("v", [B_PER_CORE, H, D, 1], f32, kind="ExternalInput")
    alpha_ext = nc.dram_tensor("alpha", [B_PER_CORE, H, 1, 1], f32, kind="ExternalInput")
    eta_ext = nc.dram_tensor("eta", [B_PER_CORE, H, 1, 1], f32, kind="ExternalInput")
    out_ext = nc.dram_tensor("out", [B_PER_CORE, H, D, D], f32, kind="ExternalOutput")

    NP = PAIRS_PER_CORE
    p4 = slice(0, P, 32)  # partitions {0,32,64,96}

    with tile.TileContext(nc) as tc:
        with (
            tc.tile_pool(name="const", bufs=1) as const_pool,
            tc.tile_pool(name="m_in", bufs=4) as m_pool,
            tc.tile_pool(name="tmp", bufs=2) as tmp_pool,
            tc.tile_pool(name="ms", bufs=2) as ms_pool,
            tc.tile_pool(name="outer", bufs=2) as outer_pool,
            tc.tile_pool(name="outt", bufs=3) as out_pool,
            tc.tile_pool(name="kb", bufs=3) as kb_pool,
            tc.tile_pool(name="scratch", bufs=2) as scratch_pool,
            tc.tile_pool(name="small", bufs=3) as small_pool,
        ):
            # --- one-time: broadcast alpha (cols 0:NP) and eta (cols NP:2NP) ---
            ab4 = const_pool.tile([P, 2 * NP], f32)
            nc.vector.memset(ab4[:], 0.0)
            nc.sync.dma_start(
                ab4[p4, 0:NP],
                alpha_ext[:].flatten().rearrange("(o c) -> o c", o=1).broadcast_to((4, NP)),
            )
            nc.sync.dma_start(
                ab4[p4, NP : 2 * NP],
                eta_ext[:].flatten().rearrange("(o c) -> o c", o=1).broadcast_to((4, NP)),
            )
            ab_bc = const_pool.tile([P, 2 * NP], f32)
            nc.vector.stream_shuffle(ab_bc[:], ab4[:], BCAST32)

            # Long-lived ping-pong buffers for the k landing pad: the per-pair
            # DMA writes only partitions {0,32,64,96}, and the shuffle reads a
            # full-[128] AP, so the tiles must stay initialized across pairs.
            kb4_tiles = []
            for i in range(2):
                t = const_pool.tile([P, D], f32, tag=f"kb4_{i}")
                nc.vector.memset(t[:], 0.0)
                kb4_tiles.append(t)

            # --- main loop over (b, h) pairs ---
            for p in range(NP):
                b, h = divmod(p, H)
                cs = lambda c: slice(c * D, (c + 1) * D)
                alpha_ap = ab_bc[:, p : p + 1]
                eta_ap = ab_bc[:, NP + p : NP + p + 1]

                m_in = m_pool.tile([P, C * D], f32)
                # big DMAs ride gpsimd's SWDGE: the SP sequencer saturates at
                # ~900ns per dma_start if it issues all four per pair
                nc.gpsimd.dma_start(
                    m_in[:], mem_ext[b, h].rearrange("(p c) j -> p (c j)", p=P)
                )
                kb4 = kb4_tiles[p % 2]
                nc.sync.dma_start(
                    kb4[p4, :],
                    k_ext[b, h].flatten().rearrange("(o j) -> o j", o=1).broadcast_to((4, D)),
                )
                v_pc = small_pool.tile([P, C], f32, tag="v_pc")
                nc.sync.dma_start(
                    v_pc[:], v_ext[b, h].flatten().rearrange("(p c) -> p c", p=P)
                )

                # k broadcast to all partitions
                k_bc = kb_pool.tile([P, D], f32, tag="k_bc")
                nc.vector.stream_shuffle(k_bc[:], kb4[:], BCAST32)

                # Ms = alpha * M  (ACT, batched, per-partition scale AP)
                ms = ms_pool.tile([P, C * D], f32)
                nc.scalar.activation(ms[:], m_in[:], Copy, scale=alpha_ap)

                # tmp = M (*) k  (DVE, one batched op; k broadcast along chunks)
                tmp = tmp_pool.tile([P, C * D], f32)
                nc.vector.tensor_tensor(
                    tmp[:].rearrange("p (c j) -> p c j", c=C),
                    m_in[:].rearrange("p (c j) -> p c j", c=C),
                    k_bc[:].rearrange("p (o j) -> p o j", o=1).broadcast_to((P, C, D)),
                    mult,
                )

                # pred[:, c] = row-sum of tmp chunk c  (ACT accumulate)
                pred = small_pool.tile([P, C], f32, tag="pred")
                trash = scratch_pool.tile([P, D], f32, tag="trash")
                for c in range(C):
                    nc.scalar.activation(
                        trash[:], tmp[:, cs(c)], Copy, accum_out=pred[:, c : c + 1]
                    )

                # s_err = eta * (pred - v)
                terr = small_pool.tile([P, C], f32, tag="terr")
                nc.vector.tensor_tensor(terr[:], pred[:], v_pc[:], subtract)
                s_err = small_pool.tile([P, C], f32, tag="s_err")
                nc.vector.tensor_scalar_mul(s_err[:], terr[:], eta_ap)

                # outer[:, c, :] = k_bc * s_err[:, c]
                outer = outer_pool.tile([P, C * D], f32)
                for c in range(C):
                    nc.vector.tensor_scalar_mul(
                        outer[:, cs(c)], k_bc[:], s_err[:, c : c + 1]
                    )

                # out = Ms + outer  (GPSIMD, one batched op)
                out_t = out_pool.tile([P, C * D], f32)
                nc.gpsimd.tensor_tensor(out_t[:], ms[:], outer[:], add)

                nc.sync.dma_start(
                    out_ext[b, h].rearrange("(p c) j -> p (c j)", p=P), out_t[:]
                )

    # Raw Bass (no Bacc.compile) skips the InstISA byte-encoding pass; without
    # it walrus fails with "ISA wrong length" on extended instructions.
    mybir.codegen_inst_isa_subclasses(nc)
    _legalize_sync_waits(nc, mybir)
    return nc


def _get_program():
    if "nc" not in _CACHE:
        _CACHE["nc"] = _build_program()
    return _CACHE["nc"]


def _run(in_maps, **kwargs):
    from concourse.bass_utils import run_bass_kernel_spmd

    nc = _get_program()
    return run_bass_kernel_spmd(nc, in_maps, list(range(N_CORES)), **kwargs)


def _make_in_maps(memory, k, v, alpha, eta):
    def prep(x):
        return np.ascontiguousarray(np.asarray(x, dtype=np.float32))

    memory, k, v, alpha, eta = map(prep, (memory, k, v, alpha, eta))
    in_maps = []
    for i in range(N_CORES):
        s = slice(i * B_PER_CORE, (i + 1) * B_PER_CORE)
        in_maps.append(
            {
                "memory": memory[s],
                "k": k[s],
                "v": v[s],
                "alpha": alpha[s],
                "eta": eta[s],
            }
        )
    return in_maps


def kernel(memory, k, v, alpha, eta):
    res = _run(_make_in_maps(memory, k, v, alpha, eta))
    return np.concatenate(
        [res.results[i]["out"] for i in range(N_CORES)], axis=0
    )
